# revision 1
# baseline (speedup 1.0000x reference)
"""Trainium2 Bass kernel: causal multi-head attention with RoPE.

Model: B=2, S=2048, D=2048, H=16 heads, head_dim=128, fp32.

Sharding (8 cores): batch (2) x head-groups (4 heads each).  Each core
computes q/k/v projections for its 4 heads, head-local attention, and a
partial output projection (row-slice of wo); the host sums the 4 partials
per batch (the tensor-parallel all-reduce done on host).

Device-side layout trick: q and k are produced directly in transposed
[head_dim, seq] layout by using the weight tile as the stationary matmul
operand.  Scores are computed transposed ([k, q]) so that:
  - the softmax denominator is a ones-vector matmul on the PE (partition
    direction sum), accumulated across k-chunks in PSUM;
  - P @ V needs no transpose (V in natural [k, head_dim] layout is the
    stationary operand, exp(scores^T) the moving one), producing the
    attention output directly in [head_dim, seq] layout;
  - that output feeds the wo matmul directly as the stationary operand.
RoPE pairs (even/odd feature columns) are made contiguous halves by
permuting wq/wk columns on the host, so the on-chip rotation is plain
half-tile elementwise ops.  Softmax is computed without max-subtraction
(scores are O(6) for this problem size/scale, exp is safe in fp32).
qT and kT spill to DRAM scratch between projection and attention phases to fit
SBUF; everything else stays resident.
"""

import math
import os
import sys

import numpy as np

for _p in ("/opt/trn_rl_repo", "/root/.axon_site/_ro/trn_rl_repo"):
    if os.path.isdir(_p) and _p not in sys.path:
        sys.path.insert(0, _p)

import concourse.bacc as bacc
import concourse.mybir as mybir
from concourse import tile
from concourse.bass_utils import run_bass_kernel_spmd

F32 = mybir.dt.float32
F32R = mybir.dt.float32r

B, S, D, H, HD = 2, 2048, 2048, 16, 128
NCORES = 8
HPC = 4          # heads per core
HGRP = NCORES // B  # head groups (4)
FPC = HPC * HD   # features per core (512)
T5 = S // 512    # number of 512-wide seq tiles
DC = D // 128    # number of 128-deep contraction chunks
SC = 1.0 / math.sqrt(HD)

# Use the PE's reduced-precision fp32 mode (1.5 cyc/row vs 2.0) when True.
# All matmul operands (and their producers) are declared float32r end-to-end,
# as the BIR verifier requires; float32r maps to np.float32 on the host.
USE_F32R = True


def _build_program(mode, f32r=USE_F32R):
    """Trace the single-core SPMD program.  mode: 'causal' | 'dense' | 'general'."""
    nc = bacc.Bacc("TRN2", target_bir_lowering=False, debug=False,
                   num_devices=NCORES)
    MDT = F32R if f32r else F32

    xT = nc.dram_tensor("xT", [D, S], MDT, kind="ExternalInput")
    wq = nc.dram_tensor("wq", [D, FPC], MDT, kind="ExternalInput")
    wk = nc.dram_tensor("wk", [D, FPC], MDT, kind="ExternalInput")
    wv = nc.dram_tensor("wv", [D, FPC], MDT, kind="ExternalInput")
    wo = nc.dram_tensor("wo", [FPC, D], MDT, kind="ExternalInput")
    cosT = nc.dram_tensor("cosT", [HD // 2, S], F32, kind="ExternalInput")
    sinT = nc.dram_tensor("sinT", [HD // 2, S], F32, kind="ExternalInput")
    ones_d = nc.dram_tensor("ones_d", [128, 1], MDT, kind="ExternalInput")
    if mode == "causal":
        m01 = nc.dram_tensor("m01", [4, 128, 512], MDT, kind="ExternalInput")
    if mode == "general":
        maskT = nc.dram_tensor("maskT", [S, S], F32, kind="ExternalInput")
    out = nc.dram_tensor("out", [S, D], F32, kind="ExternalOutput")

    qTd = nc.dram_tensor("qTd", [HPC, 128, S], MDT)  # internal scratch
    kTd = nc.dram_tensor("kTd", [HPC, 128, S], MDT)  # internal scratch

    def nk_of(q5):
        return 4 * (q5 + 1) if mode == "causal" else DC

    with tile.TileContext(nc, pool_alloc_mode='queue') as tc:
        with (
            tc.tile_pool(name="persist", bufs=1) as pp,
            tc.tile_pool(name="ktp", bufs=1) as ktpool,
            tc.tile_pool(name="qa_ps", bufs=6, space="PSUM") as gps,
        ):
            ones = pp.tile([128, 1], MDT, tag="ones", name="ones")
            nc.sync.dma_start(ones[:], ones_d[:])
            vsb = [pp.tile([128, FPC], MDT, tag=f"v{t}", name=f"v{t}")
                   for t in range(S // 128)]

            def load_xt(sb):
                tiles = {}
                def get(t5, reload=False, interleave=None):
                    if t5 not in tiles or reload:
                        tsl = slice(t5 * 512, (t5 + 1) * 512)
                        xt = [sb.tile([128, 512], MDT, tag="xt", bufs=32,
                                      name="xt") for _ in range(DC)]
                        for dc in range(DC):
                            nc.sync.dma_start(
                                xt[dc][:], xT[dc * 128:(dc + 1) * 128, tsl])
                            if interleave is not None:
                                dst, src_ = interleave[dc]
                                nc.sync.dma_start(dst[:], src_)
                        tiles[t5] = xt
                    return tiles[t5]
                return get

            # qk weight pool opens first so its DMAs prefetch during phase V
            with (
                tc.tile_pool(name="qk_w", bufs=1) as qwp,
                tc.tile_pool(name="xt_p", bufs=2) as xp,
            ):
                get_xt_shared = load_xt(xp)
                # ---- Phase V: v projection (natural [seq, feat] layout) ----
                with (
                    tc.tile_pool(name="v_w", bufs=1) as wp,
                    tc.tile_pool(name="v_sb", bufs=2) as sb,
                ):
                    ps = gps
                    get_xt = get_xt_shared
                    wv_t = [wp.tile([128, FPC], MDT, tag=f"wv{dc}",
                                    name=f"wv{dc}") for dc in range(DC)]
                    wv_pairs = [(wv_t[dc], wv[dc * 128:(dc + 1) * 128, :])
                                for dc in range(DC)]
                    xt0 = get_xt(0, interleave=wv_pairs)
                    for t5 in range(T5):
                        xt = get_xt(t5)
                        accs = [ps.tile([128, 512], F32, tag="mm", name="vps")
                                for _ in range(4)]
                        for dc in range(DC):
                            for t in range(4):
                                nc.tensor.matmul(
                                    accs[t][:],
                                    (xt[dc][:, t * 128:(t + 1) * 128]),
                                    (wv_t[dc][:]),
                                    start=(dc == 0), stop=(dc == DC - 1))
                        for t in range(4):
                            nc.scalar.copy(vsb[t5 * 4 + t][:], accs[t][:])

                # q/k weights: prefetch behind phase V's tail
                wq_t = [qwp.tile([128, FPC], MDT, tag=f"wq{dc}",
                                 name=f"wq{dc}") for dc in range(DC)]
                wk_t = [qwp.tile([128, FPC], MDT, tag=f"wk{dc}",
                                 name=f"wk{dc}") for dc in range(DC)]
                for dc in range(DC):
                    nc.sync.dma_start(wq_t[dc][:],
                                      wq[dc * 128:(dc + 1) * 128, :])
                for dc in range(DC):
                    nc.sync.dma_start(wk_t[dc][:],
                                      wk[dc * 128:(dc + 1) * 128, :])

                # ---- Phase QK: q/k projections (transposed) + RoPE ----
                with (
                    tc.tile_pool(name="qk_sb", bufs=2) as sb,
                ):
                    ps = gps
                    get_xt = get_xt_shared
                    for t5 in [3, 2, 0, 1]:
                        tsl = slice(t5 * 512, (t5 + 1) * 512)
                        xt = get_xt(t5, reload=(t5 in (0, 1)))
                        ct = sb.tile([64, 512], F32, tag="cos", bufs=2)
                        st = sb.tile([64, 512], F32, tag="sin", bufs=2)
                        nc.sync.dma_start(ct[:], cosT[:, tsl])
                        nc.sync.dma_start(st[:], sinT[:, tsl])
                        for h in range(HPC):
                            hsl = slice(h * 128, (h + 1) * 128)
                            for w_t, dstd in ((wq_t, qTd), (wk_t, kTd)):
                                acc = ps.tile([128, 512], F32, tag="mm", name="qkps")
                                for dc in range(DC):
                                    nc.tensor.matmul(
                                        acc[:], (w_t[dc][:, hsl]),
                                        (xt[dc][:]),
                                        start=(dc == 0), stop=(dc == DC - 1))
                                # RoPE: rows 0:64 = "a" (even), 64:128 = "b"
                                a, b = acc[0:64, :], acc[64:128, :]
                                m1 = sb.tile([64, 512], F32, tag="m1", bufs=3)
                                m2 = sb.tile([64, 512], F32, tag="m2", bufs=2)
                                m3 = sb.tile([64, 512], F32, tag="m3", bufs=2)
                                m4 = sb.tile([64, 512], F32, tag="m4", bufs=2)
                                nc.vector.tensor_mul(m1[:], a, ct[:])
                                nc.vector.tensor_mul(m2[:], b, st[:])
                                nc.vector.tensor_mul(m3[:], a, st[:])
                                nc.vector.tensor_mul(m4[:], b, ct[:])
                                rt = sb.tile([128, 512], MDT, tag="rt", bufs=3)
                                nc.gpsimd.tensor_sub(rt[0:64, :], m1[:], m2[:])
                                nc.gpsimd.tensor_add(rt[64:128, :], m3[:], m4[:])
                                nc.sync.dma_start(dstd[h][:, tsl], rt[:])

            # ---- Phase A: attention; Phase W: output projection ----
            with (
                tc.tile_pool(name="at_p", bufs=1) as ap,
                tc.tile_pool(name="wo_w", bufs=1) as wp,
            ):
                attnT = [ap.tile([128, S], MDT, tag=f"aT{h}", name=f"aT{h}")
                         for h in range(HPC)]
                wo_t = [[wp.tile([128, 512], MDT, tag=f"wo{h}_{o5}",
                                 name=f"wo{h}_{o5}")
                         for o5 in range(4)] for h in range(HPC)]
                with (
                    tc.tile_pool(name="a_sb", bufs=2) as sb,
                ):
                    ps = gps
                    if mode == "causal":
                        m01_t = [sb.tile([128, 512], MDT, tag=f"m01_{r}",
                                         bufs=1, name=f"m01_{r}")
                                 for r in range(4)]
                        for r in range(4):
                            nc.sync.dma_start(m01_t[r][:], m01[r])
                    for h in range(HPC):
                        kt = ktpool.tile([128, S], MDT, tag="kt", bufs=1,
                                         name="kt")
                        nc.sync.dma_start(kt[:], kTd[h][:, :])
                        for q5 in range(T5):
                            qsl = slice(q5 * 512, (q5 + 1) * 512)
                            nk = nk_of(q5)
                            qt = ktpool.tile([128, 512], MDT, tag="qt",
                                             bufs=3, name="qt")
                            nc.sync.dma_start(qt[:], qTd[h][:, qsl])
                            aps = ps.tile([128, 512], F32, tag="acc", bufs=2,
                                          name="aps")
                            dps = ps.tile([1, 512], F32, tag="acc", bufs=2,
                                          name="dps")
                            for kc in range(nk):
                                sps = ps.tile([128, 512], F32, tag="mm",
                                              bufs=6, name="sps")
                                nc.tensor.matmul(
                                    sps[:],
                                    (kt[:, kc * 128:(kc + 1) * 128]),
                                    (qt[:]),
                                    start=True, stop=True)
                                e = sb.tile([128, 512], MDT, tag="e", bufs=18)
                                r = kc - (nk - 4)
                                if mode == "causal" and r >= 0:
                                    nc.scalar.activation(
                                        e[:], sps[:],
                                        mybir.ActivationFunctionType.Exp,
                                        scale=SC)
                                    nc.vector.tensor_mul(e[:], e[:],
                                                         m01_t[r][:])
                                elif mode == "general":
                                    g = sb.tile([128, 512], F32, tag="gm",
                                                bufs=3)
                                    nc.sync.dma_start(
                                        g[:],
                                        maskT[kc * 128:(kc + 1) * 128, qsl])
                                    sm = sb.tile([128, 512], F32, tag="sm",
                                                 bufs=3)
                                    nc.vector.tensor_add(sm[:], sps[:], g[:])
                                    nc.scalar.activation(
                                        e[:], sm[:],
                                        mybir.ActivationFunctionType.Exp,
                                        scale=SC)
                                else:
                                    nc.scalar.activation(
                                        e[:], sps[:],
                                        mybir.ActivationFunctionType.Exp,
                                        scale=SC)
                                nc.tensor.matmul(
                                    dps[:], (ones[:]), (e[:]),
                                    start=(kc == 0), stop=(kc == nk - 1))
                                nc.tensor.matmul(
                                    aps[:],
                                    (vsb[kc][:, h * 128:(h + 1) * 128]),
                                    (e[:]),
                                    start=(kc == 0), stop=(kc == nk - 1))
                            r1 = sb.tile([1, 512], F32, tag="r1", bufs=3)
                            nc.vector.reciprocal(r1[:], dps[:])
                            rb = sb.tile([128, 512], F32, tag="rb", bufs=3)
                            nc.gpsimd.partition_broadcast(rb[:], r1[:])
                            nc.vector.tensor_mul(attnT[h][:, qsl], aps[:],
                                                 rb[:])
                        if h == 0:
                            for hh in range(HPC):
                                for o5 in range(4):
                                    nc.sync.dma_start(
                                        wo_t[hh][o5][:],
                                        wo[hh * 128:(hh + 1) * 128,
                                           o5 * 512:(o5 + 1) * 512])

                # ---- Phase W ----
                with (
                    tc.tile_pool(name="w_sb", bufs=2) as sb,
                ):
                    ps = gps
                    for tt in range(S // 128):
                        for o5 in range(4):
                            acc = ps.tile([128, 512], F32, tag="mm", name="ops")
                            for h in range(HPC):
                                nc.tensor.matmul(
                                    acc[:],
                                    (attnT[h][:, tt * 128:(tt + 1) * 128]),
                                    (wo_t[h][o5][:]),
                                    start=(h == 0), stop=(h == HPC - 1))
                            ot = sb.tile([128, 512], F32, tag="ot", bufs=6)
                            nc.scalar.copy(ot[:], acc[:])
                            nc.sync.dma_start(
                                out[tt * 128:(tt + 1) * 128,
                                    o5 * 512:(o5 + 1) * 512],
                                ot[:])

    nc.finalize()
    return nc


_PROGRAMS = {}


def _get_program(mode, f32r=None):
    if f32r is None:
        f32r = USE_F32R
    key = (mode, f32r)
    if key not in _PROGRAMS:
        _PROGRAMS[key] = _build_program(mode, f32r)
    return _PROGRAMS[key]


def _rope_perm():
    p = np.empty(HD, np.int64)
    p[: HD // 2] = np.arange(0, HD, 2)
    p[HD // 2:] = np.arange(1, HD, 2)
    return p


def _detect_mode(mask2):
    if not np.any(mask2):
        return "dense"
    iu = np.triu_indices(S, 1)
    il = np.tril_indices(S, 0)
    if not np.any(mask2[il]) and np.all(mask2[iu] <= -1.0e4):
        return "causal"
    return "general"


def _prepare_inputs(x, wq, wk, wv, wo, cos, sin, mask, start_p, seq_l):
    x = np.asarray(x, np.float32)
    wq = np.asarray(wq, np.float32)
    wk = np.asarray(wk, np.float32)
    wv = np.asarray(wv, np.float32)
    wo = np.asarray(wo, np.float32)
    cos = np.asarray(cos, np.float32)
    sin = np.asarray(sin, np.float32)
    mask2 = np.asarray(mask, np.float32).reshape(S, S)
    sp = int(np.asarray(start_p))
    sl = int(np.asarray(seq_l))
    assert sl == S, f"kernel hardcodes seq_l == {S}, got {sl}"

    mode = _detect_mode(mask2)

    c = np.ascontiguousarray(cos[sp:sp + sl].T)  # [64, S]
    s = np.ascontiguousarray(sin[sp:sp + sl].T)

    perm = _rope_perm()
    in_maps = []
    shared = {"cosT": c, "sinT": s,
              "ones_d": np.ones((128, 1), np.float32)}
    if mode == "causal":
        i = np.arange(128)[:, None]
        j = np.arange(512)[None, :]
        m01 = np.empty((4, 128, 512), np.float32)
        for r in range(4):
            m01[r] = (j >= i + 128 * r).astype(np.float32)
        shared["m01"] = m01
    if mode == "general":
        shared["maskT"] = np.ascontiguousarray(mask2.T * math.sqrt(HD))

    xTs = [np.ascontiguousarray(x[b].T) for b in range(B)]
    for core in range(NCORES):
        b = core // HGRP
        g = core % HGRP
        hs = g * HPC  # first global head of this core
        cols = []
        for h in range(HPC):
            base = (hs + h) * HD
            cols.append(base + perm)
        cols = np.concatenate(cols)
        csl = slice(hs * HD, hs * HD + FPC)
        in_maps.append({
            "xT": xTs[b],
            "wq": np.ascontiguousarray(wq[:, cols]),
            "wk": np.ascontiguousarray(wk[:, cols]),
            "wv": np.ascontiguousarray(wv[:, csl]),
            "wo": np.ascontiguousarray(wo[csl, :]),
            **shared,
        })
    return mode, in_maps


def run(inputs, trace=False):
    mode, in_maps = _prepare_inputs(**inputs)
    nc = _get_program(mode)
    res = run_bass_kernel_spmd(nc, in_maps, list(range(NCORES)), trace=trace)
    out = np.empty((B, S, D), np.float32)
    for b in range(B):
        acc = res.results[b * HGRP]["out"].astype(np.float32)
        for g in range(1, HGRP):
            acc = acc + res.results[b * HGRP + g]["out"]
        out[b] = acc
    return out, res


def kernel(**inputs):
    out, _ = run(inputs, trace=False)
    return out

